# revision 16
# baseline (speedup 1.0000x reference)
"""Trainium2 Bass kernel for nn_Attention_based_Adjacency_Matrix.

Computes, for features [n, d] and a [d, 1]:
    score[i,j]  = sum_k |f[i,k] - f[j,k]| * a[k]
    adjacency   = exp(-relu(score))
    dsq         = rowsum(adjacency) ** -0.5
    normalized  = dsq[:,None] * adjacency * dsq[None,:]
    returns (normalized, adjacency)

Strategy (8 NeuronCores, row-block sharding of the n dimension):
  * Host prescales g = f * a.T (a >= 0, so sum_k |g_i - g_j| == score exactly
    up to fp rounding) and ships g.T (replicated) + each core's row shard.
  * Per core, for a 128-row i-block and a 512-col j-chunk, PSUM accumulates
    score via 256 identity-weight matmuls:  psum += |g[j, k] - g[i, k]| with
    the abs-diff tiles [128 i x 512 j] produced one k (round r) at a time by
    DVE (custom ABS_DIFF uop) and ACT (activation Abs with bias=-x).
    The y-row (g[:, k] over the j-chunk) is partition-broadcast by DMA.
  * adjacency = exp(-psum) on ACT, row degrees reduced on DVE,
    dsq = rsqrt(degree); per-shard dsq vectors are AllGathered in-kernel;
    second pass scales adjacency into normalized.
"""

import numpy as np

import concourse.bacc as bacc
import concourse.tile as tile
from concourse import mybir
from concourse.bass_utils import run_bass_kernel_spmd
from concourse.masks import make_identity

# ---- custom DVE op: out = |in0 - s0| ---------------------------------------
import concourse.dve_ops as dve_ops
from concourse.dve_ops import DveOp
from concourse.dve_spec import Spec, Src0, C0, maxx, lower
from concourse.dve_uop import DveOpSpec

_ABS_NAME = "ABS_DIFF_ANT"


def _register_absdiff():
    if _ABS_NAME in dve_ops._SUB_OPCODE_FOR_NAME:
        return next(o for o in dve_ops.OPS if o.name == _ABS_NAME)
    spec = Spec(
        body=maxx(Src0 - C0, C0 - Src0),
        reference=lambda in0, in1, s0, s1, imm2: np.abs(
            in0.astype(np.float32) - s0
        ),
    )
    opcode = dve_ops._CUSTOM_DVE_ROW_BASE + len(dve_ops.OPS)
    shas = {}
    for ver in ("v3", "v4"):
        try:
            uops = lower(spec, ver=ver)
            shas[ver] = DveOpSpec(
                name=_ABS_NAME, opcode=opcode, uops=uops, rd1_en=False
            ).sha(ver)
        except Exception:
            pass
    assert shas, "lower() failed for all vers"
    op = DveOp(_ABS_NAME, spec, subdim=False, uops_sha=shas)
    dve_ops.OPS.append(op)
    dve_ops.CUSTOM_DVE_SPECS[_ABS_NAME] = op.spec
    dve_ops._SUB_OPCODE_FOR_NAME[_ABS_NAME] = opcode
    assert max(dve_ops._SUB_OPCODE_FOR_NAME.values()) < 0x20
    return op


ABS_DIFF_ANT = _register_absdiff()

_PAIR_NAME = "ABS_DIFF_PAIR_ANT"


def _register_absdiff_pair():
    """out = |in0 - s0| + |in1 - s1| — two k-rounds fused in one DVE pass."""
    if _PAIR_NAME in dve_ops._SUB_OPCODE_FOR_NAME:
        return next(o for o in dve_ops.OPS if o.name == _PAIR_NAME)
    from concourse.dve_spec import Src1, C1

    spec = Spec(
        body=maxx(Src0 - C0, C0 - Src0) + maxx(Src1 - C1, C1 - Src1),
        reference=lambda in0, in1, s0, s1, imm2: (
            np.abs(in0.astype(np.float32) - s0) + np.abs(in1.astype(np.float32) - s1)
        ),
    )
    opcode = dve_ops._CUSTOM_DVE_ROW_BASE + len(dve_ops.OPS)
    shas = {}
    for ver in ("v3", "v4"):
        try:
            uops = lower(spec, ver=ver)
            shas[ver] = DveOpSpec(
                name=_PAIR_NAME, opcode=opcode, uops=uops, rd1_en=True
            ).sha(ver)
        except Exception:
            pass
    assert shas, "lower() failed for all vers"
    op = DveOp(_PAIR_NAME, spec, subdim=False, uops_sha=shas)
    dve_ops.OPS.append(op)
    dve_ops.CUSTOM_DVE_SPECS[_PAIR_NAME] = op.spec
    dve_ops._SUB_OPCODE_FOR_NAME[_PAIR_NAME] = opcode
    assert max(dve_ops._SUB_OPCODE_FOR_NAME.values()) < 0x20
    return op


ABS_DIFF_PAIR_ANT = _register_absdiff_pair()

f32 = mybir.dt.float32
P = 128  # partitions / i-block size
JC = 512  # j-chunk (max f32 moving free dim / one PSUM bank)
RB = 8  # k-rounds per broadcast-DMA batch


def build_kernel(n, d, ncores, acts_pattern=(4, 2, 2, 2, 4, 2, 2, 2),
                 no_cc=False, precision="f32r"):
    """Build + compile the per-core SPMD program.

    acts_pattern[b]: of each RB-round DMA batch, how many rounds ACT handles
    as single abs tiles for i-block b; the rest are fused pairwise on DVE.
    Entries must be even. precision: "f32r" (tf32-rounded tiles, full-rate
    PE) or "f32" (exact fp32 tiles, quarter-rate PE).
    """
    rows = n // ncores  # rows per core
    ib = rows // P  # i-blocks per core
    njc = n // JC  # j-chunks
    nrb = d // RB  # DMA batches per j-chunk
    assert rows % P == 0 and n % JC == 0 and d % RB == 0

    nc = bacc.Bacc(None, num_devices=ncores)
    gt = nc.dram_tensor("gt", [d, n], f32, kind="ExternalInput")
    gsh = nc.dram_tensor("gsh", [P, ib * d], f32, kind="ExternalInput")
    adjb = nc.dram_tensor("adjb", [rows, n], f32, kind="ExternalOutput")
    normb = nc.dram_tensor("normb", [rows, n], f32, kind="ExternalOutput")
    dsql = nc.dram_tensor("dsql", [rows], f32)
    dsqf = nc.dram_tensor("dsqf", [n], f32, addr_space="Shared")

    with tile.TileContext(nc) as tc:
        with (
            tc.tile_pool(name="const", bufs=1) as const,
            tc.tile_pool(name="yrep", bufs=3) as yrep_pool,
            tc.tile_pool(name="tp", bufs=20) as t_pool,
            tc.tile_pool(name="ap", bufs=8) as a_pool,
            tc.tile_pool(name="p2a", bufs=8) as p2a_pool,
            tc.tile_pool(name="p2n", bufs=4) as p2n_pool,
            tc.tile_pool(name="psum", bufs=8, space="PSUM") as psum_pool,
        ):
            tdt = mybir.dt.float32r if precision == "f32r" else f32
            ident0 = const.tile([P, P], f32)
            make_identity(nc, ident0[:])
            ident = const.tile([P, P], tdt)
            nc.scalar.copy(ident[:], ident0[:])  # ACT-funnel for matmul deps

            # x shard: [P, ib, d]; xs[p, b, k] = g[b*P + p, k] (this core's rows)
            xs = const.tile([P, ib, d], f32)
            nc.sync.dma_start(xs[:], gsh[:].rearrange("p (b k) -> p b k", b=ib))
            nxs = const.tile([P, ib, d], f32)
            nc.vector.tensor_scalar_mul(nxs[:], xs[:], -1.0)

            rs_all = const.tile([P, ib, njc], f32)  # row-sum partials
            dsq_my = const.tile([P, ib], f32)

            # ---------------- phase 1: score -> adjacency -> degrees --------
            for jc in range(njc):
                js = slice(jc * JC, (jc + 1) * JC)
                ps = [psum_pool.tile([P, JC], f32, name="ps", tag="ps") for _ in range(ib)]
                for rb in range(nrb):
                    yb = yrep_pool.tile([P, RB, JC], f32)
                    nc.sync.dma_start(
                        yb[:],
                        gt[rb * RB : (rb + 1) * RB, js]
                        .rearrange("(o k) j -> o k j", o=1)
                        .to_broadcast((P, RB, JC)),
                    )
                    # rounds [0, acts_pattern[b]) -> ACT single-abs tiles;
                    # the rest -> DVE fused pair tiles. Round 0 on ACT also
                    # keeps the start-matmul's deps on one (ACT) semaphore:
                    # psum release (exp) + t-producer.
                    for rl in range(RB):
                        r = rb * RB + rl
                        for b in range(ib):
                            acts = acts_pattern[b % len(acts_pattern)]
                            if rl >= acts and (rl - acts) % 2 == 1:
                                continue  # consumed by the pair op below
                            pair = rl >= acts
                            t = t_pool.tile([P, JC], tdt, name="t", tag="t")
                            if pair:
                                nc.vector._custom_dve(
                                    ABS_DIFF_PAIR_ANT,
                                    out=t[:],
                                    in0=yb[:, rl, :],
                                    in1=yb[:, rl + 1, :],
                                    s0=xs[:, b, r : r + 1],
                                    s1=xs[:, b, r + 1 : r + 2],
                                )
                            else:
                                nc.scalar.activation(
                                    out=t[:],
                                    in_=yb[:, rl, :],
                                    func=mybir.ActivationFunctionType.Abs,
                                    bias=nxs[:, b, r : r + 1],
                                    scale=1.0,
                                )
                            nc.tensor.matmul(
                                ps[b][:],
                                ident[:],
                                t[:],
                                start=(r == 0),
                                stop=(r + (2 if pair else 1) == d),
                            )
                for b in range(ib):
                    a_t = a_pool.tile([P, JC], f32)
                    nc.scalar.activation(
                        out=a_t[:],
                        in_=ps[b][:],
                        func=mybir.ActivationFunctionType.Exp,
                        scale=-1.0,
                    )
                    nc.sync.dma_start(adjb[b * P : (b + 1) * P, js], a_t[:])
                    nc.vector.tensor_reduce(
                        out=rs_all[:, b, jc : jc + 1],
                        in_=a_t[:],
                        axis=mybir.AxisListType.X,
                        op=mybir.AluOpType.add,
                    )

            # degrees -> dsq = degree^-0.5
            deg = const.tile([P, ib], f32)
            rec = const.tile([P, ib], f32)
            nc.vector.tensor_reduce(
                out=deg[:],
                in_=rs_all[:],
                axis=mybir.AxisListType.X,
                op=mybir.AluOpType.add,
            )
            nc.vector.reciprocal(rec[:], deg[:])
            nc.scalar.sqrt(dsq_my[:], rec[:])
            nc.sync.dma_start(dsql[:].rearrange("(b p) -> p b", p=P), dsq_my[:])

            # ---------------- all-gather degrees ----------------------------
            if no_cc:
                # single-core timing/sim variant: pretend the gather is a copy
                # (numerically wrong for ncores>1; timing-equivalent)
                for c in range(ncores):
                    nc.sync.dma_start(dsqf[c * rows : (c + 1) * rows], dsql[:])
            else:
                cc = nc.gpsimd.collective_compute(
                    "AllGather",
                    mybir.AluOpType.bypass,
                    replica_groups=[list(range(ncores))],
                    ins=[dsql[:]],
                    outs=[dsqf[:]],
                )

            # ---------------- phase 2: normalized ---------------------------
            dsqj_tiles = []
            for jc in range(njc):
                js = slice(jc * JC, (jc + 1) * JC)
                dsqj = const.tile([P, JC], f32, name=f"dsqj{jc}", tag=f"dsqj{jc}")
                nc.sync.dma_start(
                    dsqj[:],
                    dsqf[js].rearrange("(o j) -> o j", o=1).to_broadcast((P, JC)),
                )
                dsqj_tiles.append(dsqj)
            for b in range(ib):
                for jc in range(njc):
                    js = slice(jc * JC, (jc + 1) * JC)
                    a2 = p2a_pool.tile([P, JC], f32)
                    nc.sync.dma_start(a2[:], adjb[b * P : (b + 1) * P, js])
                    n_t = p2n_pool.tile([P, JC], f32)
                    nc.vector.scalar_tensor_tensor(
                        out=n_t[:],
                        in0=a2[:],
                        scalar=dsq_my[:, b : b + 1],
                        in1=dsqj_tiles[jc][:],
                        op0=mybir.AluOpType.mult,
                        op1=mybir.AluOpType.mult,
                    )
                    nc.sync.dma_start(normb[b * P : (b + 1) * P, js], n_t[:])

    nc.compile()
    return nc


# -------------------------------------------------------------------------
# host wrapper
# -------------------------------------------------------------------------
N, D, NCORES = 8192, 256, 8
_cache = {}
TRACE = False  # set True (e.g. from test.py) to capture an NTFF profile
LAST_RESULT = None  # BassKernelResults of the most recent kernel() call


PRECISION = "f32r"
ACTS_PATTERN = (4, 2, 2, 2, 4, 2, 2, 2)


def _get_nc(n=N, d=D, ncores=NCORES):
    key = (n, d, ncores, ACTS_PATTERN, PRECISION)
    if key not in _cache:
        _cache[key] = build_kernel(
            n, d, ncores, acts_pattern=ACTS_PATTERN, precision=PRECISION
        )
    return _cache[key]


def kernel(features: np.ndarray, a: np.ndarray):
    n, d = features.shape
    ncores = NCORES
    rows = n // ncores
    ib = rows // P

    # host prescale: g = f * a.T  (a >= 0 for this problem, so
    # |g_i - g_j| summed == sum a_k |f_i - f_j| up to fp rounding)
    g = (features.astype(np.float64) * a.astype(np.float64).T).astype(np.float32)
    gt = np.ascontiguousarray(g.T)  # [d, n]

    in_maps = []
    for c in range(ncores):
        sh = g[c * rows : (c + 1) * rows]  # [rows, d]
        # gsh[p, b*d + k] = g[c*rows + b*P + p, k]
        gsh = np.ascontiguousarray(
            sh.reshape(ib, P, d).transpose(1, 0, 2).reshape(P, ib * d)
        )
        in_maps.append({"gt": gt, "gsh": gsh})

    nc = _get_nc(n, d, ncores)
    res = run_bass_kernel_spmd(
        nc, in_maps, core_ids=list(range(ncores)), trace=TRACE
    )
    global LAST_RESULT
    LAST_RESULT = res

    adjacency = np.concatenate([r["adjb"] for r in res.results], axis=0)
    normalized = np.concatenate([r["normb"] for r in res.results], axis=0)
    return (normalized, adjacency)


if __name__ == "__main__":
    rng = np.random.default_rng(0)
    f = rng.standard_normal((N, D), dtype=np.float32)
    a = np.full((D, 1), 0.01, dtype=np.float32)
    out = kernel(f, a)
    print("ok", out[0].shape, out[1].shape)
